# revision 2
# baseline (speedup 1.0000x reference)
"""AlphaWeightedConv2d Trainium2 kernel.

Reference computation (B=32, CIN=COUT=64, H=W=112, K=3, pad=1):
    g = sigmoid(alpha[label])                     # [B, COUT]
    y = conv2d(x, W) * g[:,:,None,None] + (bias * g)[:,:,None,None]

Strategy: data-parallel over batch across 8 NeuronCores (4 samples/core).
Per core the conv is expressed as 9 shifted K=64 matmuls per 4-row output
chunk (CIN on partitions) over a DENSE row-major image in SBUF; the K=3
horizontal/vertical taps are realized by clipping the matmul output/rhs
windows at the image edges instead of zero-padding (the full-width center
tap goes first in each PSUM accumulation group so every element is
overwritten before the clipped taps accumulate).  Two samples ride in the
two 64-partition halves of each tile; even/odd output chunks map onto the
four 64x64 quadrants of the PE array (4 concurrent matmul streams).  The
dense layout makes every DMA descriptor a multi-KB contiguous run, so
input/output traffic moves at near line rate.  The sigmoid gate is
computed on host ([32,64] - negligible) and applied by the DVE/ACT
epilogue as a per-partition scale+bias while evacuating PSUM.  x is cast
to bf16 on host; output is written bf16 and upcast to f32 on host.
"""

import numpy as np
import ml_dtypes

B, CIN, COUT, H, W_SP = 32, 64, 64, 112, 112
N_CORES = 8
B_LOC = B // N_CORES          # 4 samples per core
IMG = H * W_SP                # 12544 elements per (sample, cin) image
CROWS = 4                     # output rows per chunk
NCHUNK = H // CROWS           # 28 chunks per sample pair
TAPS = [(dy, dx) for dy in range(3) for dx in range(3)]

_cached = None


def _build():
    from concourse import bacc, tile, mybir

    bf16 = mybir.dt.bfloat16
    f32 = mybir.dt.float32
    mult = mybir.AluOpType.mult
    add = mybir.AluOpType.add
    ident = mybir.ActivationFunctionType.Identity

    nc = bacc.Bacc("TRN2", target_bir_lowering=False, debug=False,
                   num_devices=N_CORES)
    x_ext = nc.dram_tensor("x", [B_LOC * CIN, H, W_SP], bf16,
                           kind="ExternalInput")
    w_ext = nc.dram_tensor("w", [128, 9 * 64], bf16, kind="ExternalInput")
    gs_ext = nc.dram_tensor("gs", [128, 4], f32, kind="ExternalInput")
    gb_ext = nc.dram_tensor("gb", [128, 4], f32, kind="ExternalInput")
    out_ext = nc.dram_tensor("out", [B_LOC * COUT, H, W_SP], bf16,
                             kind="ExternalOutput")

    # center tap first: it covers the full 4x112 window for every chunk,
    # so start=True overwrites all PSUM elements before clipped taps land
    TAP_ORDER = [(1, 1), (0, 0), (0, 1), (0, 2), (1, 0),
                 (1, 2), (2, 0), (2, 1), (2, 2)]
    # dx -> (src_col0, n_cols, dst_col0): horizontal shift via clipping
    COL = {0: (0, 111, 1), 1: (0, 112, 0), 2: (1, 111, 0)}

    # first x band split small so the first matmuls start early
    LOAD_SPLITS = [(0, 12), (12, 28), (28, 56), (56, 84), (84, 112)]
    FLUSH = {1: (0, 16), 4: (16, 40), 7: (40, 64), 10: (64, 88),
             12: (88, 104)}
    PREFETCH = {2: 0, 5: 1, 8: 2, 11: 3}   # pair-0 iter -> pair-1 band

    with tile.TileContext(nc) as tc:
        with (
            tc.tile_pool(name="wpool", bufs=1) as wpool,
            tc.tile_pool(name="xpool", bufs=2) as xpool,
            tc.tile_pool(name="opool", bufs=2) as opool,
            tc.tile_pool(name="pspool", bufs=8, space="PSUM") as pspool,
        ):
            w = wpool.tile([128, 9 * 64], bf16)
            gs = wpool.tile([128, 4], f32)
            gb = wpool.tile([128, 4], f32)

            nc.scalar.dma_start(w[:], w_ext.ap()[:])
            x0 = xpool.tile([128, IMG], bf16, tag="xt", name="x0")
            xv0 = x0[:, :].rearrange("p (s j) -> p s j", j=W_SP)
            for ra, rb in LOAD_SPLITS:
                nc.sync.dma_start(xv0[:, ra:rb, :],
                                  x_ext.ap()[0:128, ra:rb, :])
            nc.scalar.dma_start(gs[:], gs_ext.ap()[:])
            nc.scalar.dma_start(gb[:], gb_ext.ap()[:])

            # ---- PE warm-up: HAM clock gate starts at 1.2 GHz and needs
            # ~3.4us of sustained matmul activity to release to 2.4 GHz.
            # Burn prologue time on matmuls over the weight tile (values
            # irrelevant; psw is never read). ----
            psw = pspool.tile([128, CROWS * W_SP], f32, tag="ps")
            for _ in range(6):
                nc.tensor.matmul(psw[:, :], w[:, 0:128], w[:, 0:448],
                                 start=True, stop=True)

            xt = x0
            xt_next = None
            for p in range(2):  # sample pairs (2p, 2p+1)
                xv = xt[:, :].rearrange("p (s j) -> p s j", j=W_SP)
                osb = opool.tile([128, IMG], bf16, tag="osb", name=f"o{p}")

                for k in range(NCHUNK // 2):
                    c0, c1 = 2 * k, 2 * k + 1
                    psE = pspool.tile([128, CROWS * W_SP], f32, tag="ps")
                    psO = pspool.tile([128, CROWS * W_SP], f32, tag="ps")
                    # ---- 9 taps x 4 quadrant streams ----
                    for i, (dy, dx) in enumerate(TAP_ORDER):
                        st, sp = i == 0, i == 8
                        sc0, n_c, dc0 = COL[dx]
                        ti = dy * 3 + dx
                        for c, ps, swap in ((c0, psE, False), (c1, psO, True)):
                            src_lo = CROWS * c + dy - 1
                            lo = max(0, src_lo)
                            hi = min(H, src_lo + CROWS)
                            n_r = hi - lo
                            dr0 = lo - src_lo
                            ra = xv[0:64, lo:hi, sc0:sc0 + n_c]
                            rb = xv[64:128, lo:hi, sc0:sc0 + n_c]
                            aslice = ps[64:128] if swap else ps[0:64]
                            bslice = ps[0:64] if swap else ps[64:128]
                            av = aslice.rearrange(
                                "p (r j) -> p r j", j=W_SP)[
                                :, dr0:dr0 + n_r, dc0:dc0 + n_c]
                            bv = bslice.rearrange(
                                "p (r j) -> p r j", j=W_SP)[
                                :, dr0:dr0 + n_r, dc0:dc0 + n_c]
                            nc.tensor.matmul(
                                av, w[0:64, ti * 64:(ti + 1) * 64],
                                ra, start=st, stop=sp)
                            nc.tensor.matmul(
                                bv, w[64:128, ti * 64:(ti + 1) * 64],
                                rb, start=st, stop=sp)
                    # ---- epilogue: (psum * g) + bias*g while evacuating
                    #      PSUM; work split between VectorE and ScalarE ----
                    for c, ps, swap in ((c0, psE, False), (c1, psO, True)):
                        ov = osb[:, c * CROWS * W_SP:(c + 1) * CROWS * W_SP]
                        pv = ps[:, :]
                        if not swap:
                            if k % 2 == 0:
                                nc.scalar.activation(
                                    ov, pv, ident,
                                    bias=gb[:, 2 * p:2 * p + 1],
                                    scale=gs[:, 2 * p:2 * p + 1])
                            else:
                                nc.vector.tensor_scalar(
                                    ov, pv, gs[:, 2 * p:2 * p + 1],
                                    gb[:, 2 * p:2 * p + 1], mult, add)
                        else:
                            # psO: partitions 64:128 hold sample A, 0:64 B
                            nc.vector.tensor_scalar(
                                ov[0:64], pv[64:128],
                                gs[64:128, 2 * p + 1:2 * p + 2],
                                gb[64:128, 2 * p + 1:2 * p + 2], mult, add)
                            nc.scalar.activation(
                                ov[64:128], pv[0:64], ident,
                                bias=gb[0:64, 2 * p + 1:2 * p + 2],
                                scale=gs[0:64, 2 * p + 1:2 * p + 2])
                    # ---- flush finished row bands (scalar HWDGE queue so
                    #      the sync queue stays clear for loads) ----
                    if k in FLUSH:
                        ra, rb = FLUSH[k]
                        src = osb[:, ra * W_SP:rb * W_SP].rearrange(
                            "p (r j) -> p r j", j=W_SP)
                        nc.scalar.dma_start(
                            out_ext.ap()[p * 128:(p + 1) * 128, ra:rb, :],
                            src)
                    if k == 13:
                        # final flush sits on the critical tail: keep it
                        # small and split across both HWDGE queues
                        for (ra, rb), eng in (((104, 108), nc.sync),
                                              ((108, 112), nc.scalar)):
                            src = osb[:, ra * W_SP:rb * W_SP].rearrange(
                                "p (r j) -> p r j", j=W_SP)
                            eng.dma_start(
                                out_ext.ap()[p * 128:(p + 1) * 128,
                                             ra:rb, :],
                                src)
                    # spread pair-1 band loads across pair-0 compute
                    if p == 0 and k in PREFETCH:
                        b = PREFETCH[k]
                        if b == 0:
                            xt_next = xpool.tile([128, IMG], bf16,
                                                 tag="xt", name="x1")
                        xnv = xt_next[:, :].rearrange(
                            "p (s j) -> p s j", j=W_SP)
                        ra, rb = 28 * b, 28 * b + 28
                        nc.sync.dma_start(xnv[:, ra:rb, :],
                                          x_ext.ap()[128:256, ra:rb, :])
                xt = xt_next

    nc.compile()
    return nc


def _prep_inputs(x, W, bias, alpha, label):
    label = np.asarray(label).astype(np.int64)
    af = np.asarray(alpha, np.float32)
    g = 1.0 / (1.0 + np.exp(-af[label]))          # [B, COUT] f32
    gbv = g * np.asarray(bias, np.float32)[None, :]

    # weights: [128, 9*64] bf16; rows 0:64 and 64:128 both = W[cout,cin,dy,dx]
    # arranged as w64[cin, tap*64 + cout]
    wf = np.asarray(W, np.float32)                # [COUT, CIN, 3, 3]
    w64 = np.transpose(wf, (1, 2, 3, 0)).reshape(CIN, 9 * COUT)
    w128 = np.concatenate([w64, w64], axis=0).astype(ml_dtypes.bfloat16)

    xb = np.asarray(x, np.float32).astype(ml_dtypes.bfloat16)
    xb = xb.reshape(B, CIN, H, W_SP)

    in_maps = []
    for core in range(N_CORES):
        s = core * B_LOC
        gsc = np.zeros((128, 4), np.float32)
        gbc = np.zeros((128, 4), np.float32)
        for p in range(2):
            a, b = s + 2 * p, s + 2 * p + 1
            gsc[0:64, 2 * p] = g[a]
            gsc[64:128, 2 * p] = g[b]
            gsc[0:64, 2 * p + 1] = g[b]      # swapped parity
            gsc[64:128, 2 * p + 1] = g[a]
            gbc[0:64, 2 * p] = gbv[a]
            gbc[64:128, 2 * p] = gbv[b]
            gbc[0:64, 2 * p + 1] = gbv[b]
            gbc[64:128, 2 * p + 1] = gbv[a]
        in_maps.append({
            "x": np.ascontiguousarray(
                xb[s:s + B_LOC].reshape(B_LOC * CIN, H, W_SP)),
            "w": w128,
            "gs": gsc,
            "gb": gbc,
        })
    return in_maps


def kernel(x, W, bias, alpha, label):
    global _cached
    from concourse.bass_utils import run_bass_kernel_spmd

    if _cached is None:
        _cached = _build()
    nc = _cached
    in_maps = _prep_inputs(x, W, bias, alpha, label)
    res = run_bass_kernel_spmd(nc, in_maps, core_ids=list(range(N_CORES)))
    out = np.concatenate(
        [np.asarray(res.results[i]["out"], np.float32).reshape(
            B_LOC, COUT, H, W_SP) for i in range(N_CORES)], axis=0)
    return out


# revision 3
# speedup vs baseline: 1.2039x; 1.2039x over previous
"""AlphaWeightedConv2d Trainium2 kernel.

Reference computation (B=32, CIN=COUT=64, H=W=112, K=3, pad=1):
    g = sigmoid(alpha[label])                     # [B, COUT]
    y = conv2d(x, W) * g[:,:,None,None] + (bias * g)[:,:,None,None]

Strategy: data-parallel over batch across 8 NeuronCores (4 samples/core).
The host pre-pads each image with the conv zero border to [114, 114], so
the device image is a dense row-major array where every K=3 tap window of
every 4-row output chunk is a plain 2D slice: rhs = x[4c+dy : 4c+dy+4,
dx : dx+112].  All matmuls write full contiguous 448-element PSUM rows
(contiguous PSUM APs keep the PE drain at full rate) and all DMA
descriptors are multi-KB contiguous runs (near line rate).  Two samples
ride in the two 64-partition halves of each tile; even/odd output chunks
map onto the four 64x64 quadrants of the PE array (4 concurrent matmul
streams, separate PSUM banks).  The sigmoid gate is computed on host
([32,64] - negligible) and applied by the DVE/ACT epilogue as a
per-partition scale+bias while evacuating PSUM.  x is cast to bf16 on
host; output is written bf16 and upcast to f32 on host.
"""

import numpy as np
import ml_dtypes

B, CIN, COUT, H, W_SP = 32, 64, 64, 112, 112
N_CORES = 8
B_LOC = B // N_CORES          # 4 samples per core
HP = H + 2                    # 114 padded rows
WP = W_SP + 2                 # 114 padded cols
IMG = HP * WP                 # 12996 padded elements per (sample, cin)
CROWS = 4                     # output rows per chunk
NCHUNK = H // CROWS           # 28 chunks per sample pair
TAPS = [(dy, dx) for dy in range(3) for dx in range(3)]

_cached = None


def _build():
    from concourse import bacc, tile, mybir

    bf16 = mybir.dt.bfloat16
    f32 = mybir.dt.float32
    mult = mybir.AluOpType.mult
    add = mybir.AluOpType.add
    ident = mybir.ActivationFunctionType.Identity

    nc = bacc.Bacc("TRN2", target_bir_lowering=False, debug=False,
                   num_devices=N_CORES)
    x_ext = nc.dram_tensor("x", [B_LOC * CIN, HP, WP], bf16,
                           kind="ExternalInput")
    w_ext = nc.dram_tensor("w", [128, 9 * 64], bf16, kind="ExternalInput")
    gs_ext = nc.dram_tensor("gs", [128, 4], f32, kind="ExternalInput")
    gb_ext = nc.dram_tensor("gb", [128, 4], f32, kind="ExternalInput")
    out_ext = nc.dram_tensor("out", [B_LOC * COUT, H, W_SP], bf16,
                             kind="ExternalOutput")

    # row bands of the padded image; the first is small so the first
    # chunk's matmuls can start as early as possible
    LOAD_SPLITS = [(0, 10), (10, 30), (30, 58), (58, 86), (86, 114)]
    # iteration k -> output row range flushed after its epilogues
    FLUSH = {3: (0, 28), 6: (28, 56), 10: (56, 84), 12: (84, 104)}
    PREFETCH = {2: 0, 5: 1, 8: 2, 11: 3}   # pair-0 iter -> pair-1 band

    with tile.TileContext(nc) as tc:
        with (
            tc.tile_pool(name="wpool", bufs=1) as wpool,
            tc.tile_pool(name="xpool", bufs=2) as xpool,
            tc.tile_pool(name="opool", bufs=8) as opool,
            tc.tile_pool(name="pspool", bufs=8, space="PSUM") as pspool,
        ):
            w = wpool.tile([128, 9 * 64], bf16)
            gs = wpool.tile([128, 4], f32)
            gb = wpool.tile([128, 4], f32)
            wu = wpool.tile([128, 224], bf16)

            def load_pair(p, spans):
                xt = xpool.tile([128, IMG], bf16, tag="xt", name=f"x{p}")
                for ra, rb in spans:
                    nc.sync.dma_start(
                        xt[:, ra * WP:rb * WP],
                        x_ext.ap()[p * 128:(p + 1) * 128, ra:rb, :])
                return xt

            # w first on the sync queue (small), then the x bands; the
            # first real matmul needs both
            nc.sync.dma_start(w[:], w_ext.ap()[:])
            x0 = load_pair(0, LOAD_SPLITS)
            nc.scalar.dma_start(gs[:], gs_ext.ap()[:])
            nc.scalar.dma_start(gb[:], gb_ext.ap()[:])

            # ---- PE warm-up: HAM clock gate starts at 1.2 GHz and needs
            # ~3.4us of sustained matmul activity to release to 2.4 GHz.
            # The PE is idle during the prologue DMAs anyway, so burn that
            # time on zero matmuls into a scratch PSUM bank. ----
            nc.gpsimd.memset(wu[:], 0.0)
            psw = pspool.tile([128, CROWS * W_SP], f32, tag="ps")
            for _ in range(12):
                nc.tensor.matmul(psw[:, 0:224], wu[:, 0:128], wu[:, 0:224],
                                 start=True, stop=True)

            xt = x0
            xt_next = None
            for p in range(2):  # sample pairs (2p, 2p+1)
                xv = xt[:, :].rearrange("p (s j) -> p s j", j=WP)
                OSB = [opool.tile([128, 7 * CROWS * W_SP], bf16, tag="osb",
                                  name=f"o{p}{t}") for t in range(4)]

                for k in range(NCHUNK // 2):
                    c0, c1 = 2 * k, 2 * k + 1
                    psE = pspool.tile([128, CROWS * W_SP], f32, tag="ps")
                    psO = pspool.tile([128, CROWS * W_SP], f32, tag="ps")
                    # ---- 9 taps x 4 quadrant streams: even chunk ->
                    # quadrants (0,0)/(64,64), odd -> (0,64)/(64,0) ----
                    for i, (dy, dx) in enumerate(TAPS):
                        st, sp = i == 0, i == 8
                        for c, ps, swap in ((c0, psE, False), (c1, psO, True)):
                            ra = xv[0:64, 4 * c + dy:4 * c + dy + 4,
                                    dx:dx + 112]
                            rb = xv[64:128, 4 * c + dy:4 * c + dy + 4,
                                    dx:dx + 112]
                            aslice = ps[64:128] if swap else ps[0:64]
                            bslice = ps[0:64] if swap else ps[64:128]
                            nc.tensor.matmul(
                                aslice.rearrange("p (r j) -> p r j", j=W_SP),
                                w[0:64, i * 64:(i + 1) * 64],
                                ra, start=st, stop=sp)
                            nc.tensor.matmul(
                                bslice.rearrange("p (r j) -> p r j", j=W_SP),
                                w[64:128, i * 64:(i + 1) * 64],
                                rb, start=st, stop=sp)
                    # ---- epilogue: (psum * g) + bias*g; work split
                    #      between VectorE and ScalarE ----
                    for c, ps, swap in ((c0, psE, False), (c1, psO, True)):
                        ov = OSB[c // 7][:, (c % 7) * CROWS * W_SP:
                                         (c % 7 + 1) * CROWS * W_SP]
                        pv = ps[:, :]
                        if not swap:
                            if k % 2 == 0:
                                nc.scalar.activation(
                                    ov, pv, ident,
                                    bias=gb[:, 2 * p:2 * p + 1],
                                    scale=gs[:, 2 * p:2 * p + 1])
                            else:
                                nc.vector.tensor_scalar(
                                    ov, pv, gs[:, 2 * p:2 * p + 1],
                                    gb[:, 2 * p:2 * p + 1], mult, add)
                        else:
                            # psO: partitions 64:128 hold sample A, 0:64 B
                            nc.vector.tensor_scalar(
                                ov[0:64], pv[64:128],
                                gs[64:128, 2 * p + 1:2 * p + 2],
                                gb[64:128, 2 * p + 1:2 * p + 2], mult, add)
                            nc.scalar.activation(
                                ov[64:128], pv[0:64], ident,
                                bias=gb[0:64, 2 * p + 1:2 * p + 2],
                                scale=gs[0:64, 2 * p + 1:2 * p + 2])
                    # ---- flush finished row bands (scalar HWDGE queue so
                    #      the sync queue stays clear for loads) ----
                    if k in FLUSH:
                        ra, rb = FLUSH[k]
                        t = ra // 28
                        src = OSB[t][:, (ra - 28 * t) * W_SP:
                                     (rb - 28 * t) * W_SP].rearrange(
                            "p (r j) -> p r j", j=W_SP)
                        nc.scalar.dma_start(
                            out_ext.ap()[p * 128:(p + 1) * 128, ra:rb, :],
                            src)
                    if k == 13:
                        # final flush sits on the critical tail: keep it
                        # small and split across both HWDGE queues
                        for (ra, rb), eng in (((104, 108), nc.sync),
                                              ((108, 112), nc.scalar)):
                            src = OSB[3][:, (ra - 84) * W_SP:
                                         (rb - 84) * W_SP].rearrange(
                                "p (r j) -> p r j", j=W_SP)
                            eng.dma_start(
                                out_ext.ap()[p * 128:(p + 1) * 128,
                                             ra:rb, :],
                                src)
                    # spread pair-1 band loads across pair-0 compute
                    if p == 0 and k in PREFETCH:
                        b = PREFETCH[k]
                        if b == 0:
                            xt_next = xpool.tile([128, IMG], bf16,
                                                 tag="xt", name="x1")
                        bands = [(0, 30), (30, 58), (58, 86), (86, 114)]
                        ra, rb = bands[b]
                        nc.sync.dma_start(
                            xt_next[:, ra * WP:rb * WP],
                            x_ext.ap()[128:256, ra:rb, :])
                xt = xt_next

    nc.compile()
    return nc


def _prep_inputs(x, W, bias, alpha, label):
    label = np.asarray(label).astype(np.int64)
    af = np.asarray(alpha, np.float32)
    g = 1.0 / (1.0 + np.exp(-af[label]))          # [B, COUT] f32
    gbv = g * np.asarray(bias, np.float32)[None, :]

    # weights: [128, 9*64] bf16; rows 0:64 and 64:128 both = W[cout,cin,dy,dx]
    # arranged as w64[cin, tap*64 + cout]
    wf = np.asarray(W, np.float32)                # [COUT, CIN, 3, 3]
    w64 = np.transpose(wf, (1, 2, 3, 0)).reshape(CIN, 9 * COUT)
    w128 = np.concatenate([w64, w64], axis=0).astype(ml_dtypes.bfloat16)

    xb = np.asarray(x, np.float32).astype(ml_dtypes.bfloat16)
    xb = xb.reshape(B, CIN, H, W_SP)
    # conv zero border baked in on host: [B, CIN, 114, 114]
    xp = np.zeros((B, CIN, HP, WP), dtype=ml_dtypes.bfloat16)
    xp[:, :, 1:1 + H, 1:1 + W_SP] = xb

    in_maps = []
    for core in range(N_CORES):
        s = core * B_LOC
        gsc = np.zeros((128, 4), np.float32)
        gbc = np.zeros((128, 4), np.float32)
        for p in range(2):
            a, b = s + 2 * p, s + 2 * p + 1
            gsc[0:64, 2 * p] = g[a]
            gsc[64:128, 2 * p] = g[b]
            gsc[0:64, 2 * p + 1] = g[b]      # swapped parity
            gsc[64:128, 2 * p + 1] = g[a]
            gbc[0:64, 2 * p] = gbv[a]
            gbc[64:128, 2 * p] = gbv[b]
            gbc[0:64, 2 * p + 1] = gbv[b]
            gbc[64:128, 2 * p + 1] = gbv[a]
        in_maps.append({
            "x": np.ascontiguousarray(
                xp[s:s + B_LOC].reshape(B_LOC * CIN, HP, WP)),
            "w": w128,
            "gs": gsc,
            "gb": gbc,
        })
    return in_maps


def kernel(x, W, bias, alpha, label):
    global _cached
    from concourse.bass_utils import run_bass_kernel_spmd

    if _cached is None:
        _cached = _build()
    nc = _cached
    in_maps = _prep_inputs(x, W, bias, alpha, label)
    res = run_bass_kernel_spmd(nc, in_maps, core_ids=list(range(N_CORES)))
    out = np.concatenate(
        [np.asarray(res.results[i]["out"], np.float32).reshape(
            B_LOC, COUT, H, W_SP) for i in range(N_CORES)], axis=0)
    return out
